# revision 1
# baseline (speedup 1.0000x reference)
"""Graph-transformer layer (GTLayer) on 8 Trainium2 NeuronCores.

Strategy (node-parallel over destination nodes, degree-balanced binning):
  - Host: bin the 50000 nodes into 392 blocks of <=128 nodes with nearly
    equal total degree (snake deal over degree-sorted nodes), 49 blocks
    per core. Each block's edges are packed into t_b tiles of 128 edge
    slots (pad slots have lloc -1, dropped by the one-hot scatter).
  - Edges within a block are split into a "lo" section (source col <
    32768, tiles [0, tlo)) and a "hi" section (col >= 32768, tiles
    [tlo, t_b)) so wide int16-indexed SWDGE dma_gather ops can fetch a
    whole section per instruction (3 gathers per block instead of 2 DMAs
    per tile); section sizes are global so the SPMD program is uniform.
  - Device, per core: compute q for its local nodes (49 blocks), then per
    block gather the block's edge data with three wide SWDGE gathers
    (raw col embeds lo/hi + q rows), project k|v on the fly with one fused
    [128,256] fp32r matmul per 128-edge tile, compute per-head attention
    with 4-tile-wide DVE ops, and scatter-add [weighted-v | exp-weight]
    into a per-block PSUM accumulator via one-hot matmuls.
  - Finalize per block: divide by (exp-sum + 1e-8), add residual,
    LayerNorm(eps=1e-6), write the block's 128 output rows.
  - Host scatters the 8 per-core outputs back to original node order.

All cores run one identical program; per-core behavior differs only
through input data (binned index arrays + local embed slices).
"""

import numpy as np

import concourse.bass as bass
import concourse.bacc as bacc
import concourse.tile as tile
from concourse import mybir
from concourse.bass_utils import run_bass_kernel_spmd
from concourse.masks import make_identity

N = 50000
E = 800000
D = 128
H = 8
HD = 16
NCORES = 8
NB = 49  # blocks of 128 dest nodes per core
NBP = NB * 128  # padded local nodes per core
NBLK = NCORES * NB  # blocks total
BW = 4  # edge tiles per compute batch
LOHI = 32768  # int16-indexable table row limit

F32 = mybir.dt.float32
F32R = mybir.dt.float32r
I16 = mybir.dt.int16
BF16 = mybir.dt.bfloat16


def _bcast_inner(ap: bass.AP, k: int) -> bass.AP:
    """View [..., m] AP as [..., m, k] with the inner dim broadcast."""
    return bass.AP(tensor=ap.tensor, offset=ap.offset, ap=[*ap.ap, [0, k]])


GCHUNK = 8  # max tiles per dma_gather: the HW SWDGE ring holds 1024 descs


def build_program(t_b: int, tlo: int, reps: int = 1) -> bass.Bass:
    thi = t_b - tlo
    nc = bacc.Bacc(None, num_swdge_queues=4)

    embeds = nc.dram_tensor("embeds", [N, D], F32, kind="ExternalInput")
    emb_local = nc.dram_tensor("emb_local", [NBP, D], F32, kind="ExternalInput")
    qT = nc.dram_tensor("qT", [D, D], F32, kind="ExternalInput")
    kT = nc.dram_tensor("kT", [D, D], F32, kind="ExternalInput")
    vT = nc.dram_tensor("vT", [D, D], F32, kind="ExternalInput")
    lnsc = nc.dram_tensor("lnsc", [D], F32, kind="ExternalInput")
    lnb = nc.dram_tensor("lnb", [D], F32, kind="ExternalInput")
    # merged per-block int16 row: [lloc(bf16 bits) t_b | ilo tlo*8 | ihi thi*8 | iq t_b*8]
    wm = t_b + tlo * 8 + thi * 8 + t_b * 8
    idxs_d = nc.dram_tensor("idxs", [NBP, wm], I16, kind="ExternalInput")

    qNodes = nc.dram_tensor("qNodes", [NBP, D], F32)
    out_d = nc.dram_tensor("out", [NBP, D], F32, kind="ExternalOutput")

    emb_lo = embeds[0:LOHI, :] if N > LOHI else embeds[:]
    emb_hi = embeds[LOHI:N, :] if N > LOHI else None

    with tile.TileContext(nc) as tc:
        with tc.tile_pool(name="singles", bufs=1) as singles:
            # ---- one-time constants ----
            ident_g = singles.tile([128, 128], F32)
            make_identity(nc, ident_g)
            ident = singles.tile([128, 128], F32)
            nc.vector.tensor_copy(ident[:], ident_g[:])

            iota_i = singles.tile([128, BW, 128], mybir.dt.int32)
            nc.gpsimd.iota(
                iota_i[:], pattern=[[0, BW], [1, 128]], base=0, channel_multiplier=0
            )
            iota_f = singles.tile([128, BW, 128], BF16)
            nc.vector.tensor_copy(iota_f[:], iota_i[:])

            # all local embeds resident in SBUF: [p, b, d] <- emb_local[b*128+p, d]
            embL = singles.tile([128, NB, 128], F32)
            nc.sync.dma_start(
                out=embL[:],
                in_=bass.AP(
                    tensor=emb_local,
                    offset=0,
                    ap=[[128, 128], [128 * 128, NB], [1, 128]],
                ),
            )
            # all per-block merged idx rows resident in SBUF: [p, n, wm]
            mrgL = singles.tile([128, NB, wm], I16)
            nc.sync.dma_start(
                out=mrgL[:],
                in_=bass.AP(
                    tensor=idxs_d,
                    offset=0,
                    ap=[[wm, 128], [128 * wm, NB], [1, wm]],
                ),
            )

            lnsc_t = singles.tile([128, 128], F32)
            nc.sync.dma_start(
                out=lnsc_t[:],
                in_=bass.AP(tensor=lnsc, offset=0, ap=[[0, 128], [1, 128]]),
            )
            lnb_t = singles.tile([128, 128], F32)
            nc.sync.dma_start(
                out=lnb_t[:],
                in_=bass.AP(tensor=lnb, offset=0, ap=[[0, 128], [1, 128]]),
            )
            eps_t = singles.tile([128, 1], F32)
            nc.vector.memset(eps_t[:], 1e-6)
            eps8_t = singles.tile([128, 1], F32)
            nc.vector.memset(eps8_t[:], 1e-8)

            qT_t = singles.tile([128, 128], F32)
            nc.sync.dma_start(qT_t[:], qT[:])
            kvT_raw = singles.tile([128, 256], F32)
            nc.sync.dma_start(kvT_raw[:, 0:128], kT[:])
            nc.sync.dma_start(kvT_raw[:, 128:256], vT[:])
            kvT_t = singles.tile([128, 256], F32R)
            nc.vector.tensor_copy(kvT_t[:], kvT_raw[:])

            for _rep in range(reps):
                # ---- phase A: local q table ----
                with tc.tile_pool(name="tA", bufs=3) as tA, tc.tile_pool(
                    name="psA", bufs=2, space="PSUM"
                ) as psA:
                    for b in range(NB):
                        tp = psA.tile([128, 128], F32)
                        nc.tensor.transpose(
                            out=tp[:], in_=embL[:, b, :], identity=ident[:]
                        )
                        embT = tA.tile([128, 128], F32)
                        nc.scalar.copy(embT[:], tp[:])
                        qp = psA.tile([128, 128], F32)
                        nc.tensor.matmul(
                            qp[:], lhsT=embT[:], rhs=qT_t[:], start=True, stop=True
                        )
                        qs = tA.tile([128, 128], F32)
                        nc.vector.tensor_copy(qs[:], qp[:])
                        nc.sync.dma_start(qNodes[b * 128 : (b + 1) * 128, :], qs[:])

                # ---- phase B: per block gather + attention + scatter ----
                with tc.tile_pool(
                    name="gat", bufs=3
                ) as gat, tc.tile_pool(name="work", bufs=4) as work, tc.tile_pool(
                    name="tpps", bufs=2, space="PSUM"
                ) as tpps, tc.tile_pool(
                    name="kvps", bufs=2, space="PSUM"
                ) as kvps, tc.tile_pool(
                    name="accps", bufs=2, space="PSUM"
                ) as accps, tc.tile_pool(name="finp", bufs=2) as finp:
                    qrr = [0]  # round-robin SWDGE queue assignment
                    for n in range(NB):
                        r0 = n * 128
                        mrg = mrgL[:, n, :]
                        lloc_t = mrg[:, 0:t_b].bitcast(BF16)
                        o1 = t_b
                        o2 = o1 + tlo * 8
                        o3 = o2 + thi * 8

                        def gather_chunks(out_tile, col0, ntiles, table, idx_ap, icol0):
                            done = 0
                            while done < ntiles:
                                w = min(GCHUNK, ntiles - done)
                                nc.gpsimd.dma_gather(
                                    out_ap=out_tile[:, col0 + done : col0 + done + w, :],
                                    in_ap=table,
                                    idxs_ap=idx_ap[
                                        :, icol0 + done * 8 : icol0 + (done + w) * 8
                                    ],
                                    num_idxs=w * 128,
                                    num_idxs_reg=w * 128,
                                    elem_size=128,
                                    queue_num=qrr[0] % 4,
                                )
                                qrr[0] += 1
                                done += w

                        e_all = gat.tile([128, t_b, 128], F32)
                        gather_chunks(e_all, 0, tlo, emb_lo, mrg, o1)
                        if thi > 0:
                            gather_chunks(e_all, tlo, thi, emb_hi, mrg, o2)
                        q_all = gat.tile([128, t_b, 128], F32)
                        gather_chunks(q_all, 0, t_b, qNodes[:], mrg, o3)

                        acc = accps.tile([128, 136], F32)

                        for t0 in range(0, t_b, BW):
                            w = min(BW, t_b - t0)
                            tp4 = tpps.tile([128, BW, 128], F32)
                            for j in range(w):
                                nc.tensor.transpose(
                                    out=tp4[:, j, :],
                                    in_=e_all[:, t0 + j, :],
                                    identity=ident[:],
                                )
                            ect = work.tile([128, BW, 128], F32R)
                            nc.scalar.copy(ect[:, 0:w, :], tp4[:, 0:w, :])

                            kv = kvps.tile([128, BW, 256], F32)
                            for j in range(w):
                                nc.tensor.matmul(
                                    kv[:, j, :],
                                    lhsT=ect[:, j, :],
                                    rhs=kvT_t[:],
                                    start=True,
                                    stop=True,
                                    skip_group_check=True,
                                )

                            qk = work.tile([128, BW, 128], F32)
                            nc.vector.tensor_tensor(
                                out=qk[:, 0:w, :],
                                in0=q_all[:, t0 : t0 + w, :],
                                in1=kv[:, 0:w, 0:128],
                                op=mybir.AluOpType.mult,
                            )
                            att = work.tile([128, BW * H], F32)
                            nc.vector.tensor_reduce(
                                out=att[:, 0 : w * H],
                                in_=qk[:, 0:w, :].rearrange("p w (h x) -> p (w h) x", h=H),
                                op=mybir.AluOpType.add,
                                axis=mybir.AxisListType.X,
                            )
                            attc = work.tile([128, BW * H], F32)
                            nc.gpsimd.tensor_scalar(
                                out=attc[:, 0 : w * H],
                                in0=att[:, 0 : w * H],
                                scalar1=10.0,
                                scalar2=-10.0,
                                op0=mybir.AluOpType.min,
                                op1=mybir.AluOpType.max,
                            )
                            expw = work.tile([128, BW * H], F32)
                            nc.scalar.activation(
                                out=expw[:, 0 : w * H],
                                in_=attc[:, 0 : w * H],
                                func=mybir.ActivationFunctionType.Exp,
                            )

                            x_t = work.tile([128, BW, 136], F32)
                            nc.vector.tensor_tensor(
                                out=x_t[:, 0:w, 0:128].rearrange(
                                    "p w (h x) -> p w h x", h=H
                                ),
                                in0=kv[:, 0:w, 128:256].rearrange(
                                    "p w (h x) -> p w h x", h=H
                                ),
                                in1=_bcast_inner(
                                    expw[:, 0 : w * H].rearrange("p (w h) -> p w h", h=H),
                                    HD,
                                ),
                                op=mybir.AluOpType.mult,
                            )
                            nc.gpsimd.tensor_copy(
                                x_t[:, 0:w, 128:136],
                                expw[:, 0 : w * H].rearrange("p (w h) -> p w h", h=H),
                            )

                            p_t = work.tile([128, BW, 128], F32)
                            nc.vector.tensor_tensor(
                                out=p_t[:, 0:w, :],
                                in0=iota_f[:, 0:w, :],
                                in1=_bcast_inner(lloc_t[:, t0 : t0 + w], 128),
                                op=mybir.AluOpType.is_equal,
                            )

                            for j in range(w):
                                nc.tensor.matmul(
                                    acc[:],
                                    lhsT=p_t[:, j, :],
                                    rhs=x_t[:, j, :],
                                    start=(t0 + j == 0),
                                    stop=(t0 + j == t_b - 1),
                                    skip_group_check=True,
                                )

                        # finalize block n
                        accs = finp.tile([128, 136], F32)
                        nc.vector.tensor_copy(accs[:], acc[:])
                        dinv = finp.tile([128, H], F32)
                        nc.vector.tensor_scalar_add(dinv[:], accs[:, 128:136], 1e-8)
                        nc.vector.reciprocal(dinv[:], dinv[:])

                        res = finp.tile([128, 128], F32)
                        nc.vector.tensor_tensor(
                            out=res[:].rearrange("p (h x) -> p h x", h=H),
                            in0=accs[:, 0:128].rearrange("p (h x) -> p h x", h=H),
                            in1=_bcast_inner(dinv[:], HD),
                            op=mybir.AluOpType.mult,
                        )
                        nc.vector.tensor_add(res[:], res[:], embL[:, n, :])

                        stats = finp.tile([128, 6], F32)
                        nc.vector.bn_stats(out=stats[:], in_=res[:])
                        mv = finp.tile([128, 2], F32)
                        nc.vector.bn_aggr(out=mv[:], in_=stats[:])

                        sd = finp.tile([128, 1], F32)
                        nc.scalar.activation(
                            out=sd[:],
                            in_=mv[:, 1:2],
                            func=mybir.ActivationFunctionType.Sqrt,
                            bias=eps_t[:],
                            scale=1.0,
                        )

                        nc.vector.reciprocal(sd[:], sd[:])

                        xm = finp.tile([128, 128], F32)
                        nc.vector.tensor_scalar_sub(xm[:], res[:], mv[:, 0:1])
                        y = finp.tile([128, 128], F32)
                        nc.vector.scalar_tensor_tensor(
                            out=y[:],
                            in0=xm[:],
                            scalar=sd[:],
                            in1=lnsc_t[:],
                            op0=mybir.AluOpType.mult,
                            op1=mybir.AluOpType.mult,
                        )
                        nc.vector.tensor_add(y[:], y[:], lnb_t[:])
                        nc.sync.dma_start(out_d[r0 : r0 + 128, :], y[:])

    nc.finalize()
    return nc


def _wrap16(idx_flat, nblk, ntiles):
    """[nblk, ntiles*128] lane-indexed indices -> [nblk*128, ntiles*8] int16
    host layout: per block a [128, ntiles*8] tile; the 16-row wrapped block
    (index i at (i % 16, i // 16)) replicated across all 8 Q7-core stripes."""
    num = ntiles * 128
    blkw = np.zeros((nblk, 16, ntiles * 8), np.int16)
    i = np.arange(num)
    blkw[:, i % 16, i // 16] = idx_flat
    return np.tile(blkw, (1, 8, 1)).reshape(nblk * 128, ntiles * 8)


def _prepare_core_inputs(embeds, edge_index, qTrans, kTrans, vTrans, ln_scale, ln_bias):
    rows = np.asarray(edge_index[0]).astype(np.int64)
    cols = np.asarray(edge_index[1]).astype(np.int64)
    n_nodes = N

    # --- degree-balanced node->block binning (snake deal) ---
    deg = np.bincount(rows, minlength=n_nodes)
    order = np.argsort(-deg, kind="stable")
    idx = np.arange(n_nodes, dtype=np.int64)
    rnd = idx // NBLK
    pos = idx % NBLK
    snake = np.where(rnd % 2 == 0, pos, NBLK - 1 - pos)
    blk = np.empty(n_nodes, dtype=np.int64)
    slot = np.empty(n_nodes, dtype=np.int64)
    blk[order] = snake
    slot[order] = rnd

    # refine: swap nodes between blocks to pull the max per-block hi/lo
    # section loads down toward the smallest tile multiple >= the mean,
    # shrinking t_b (each section is padded to its global max).
    hi_mask = np.asarray(cols) >= LOHI
    deg_hi = np.bincount(rows[hi_mask], minlength=n_nodes)
    deg_lo = deg - deg_hi
    load_hi = np.zeros(NBLK, dtype=np.int64)
    load_lo = np.zeros(NBLK, dtype=np.int64)
    np.add.at(load_hi, blk, deg_hi)
    np.add.at(load_lo, blk, deg_lo)
    for load, dsec in ((load_hi, deg_hi), (load_lo, deg_lo)):
        cap = int(np.ceil(load.mean() / 128)) * 128
        for _ in range(300):
            bmax = int(load.argmax())
            if load[bmax] <= cap:
                break
            bmin = int(load.argmin())
            na = np.where(blk == bmax)[0]
            nb = np.where(blk == bmin)[0]
            a = na[np.argmax(dsec[na])]
            b = nb[np.argmin(dsec[nb])]
            d = dsec[a] - dsec[b]
            if d <= 0:
                break
            other = deg_lo if dsec is deg_hi else deg_hi
            oload = load_lo if dsec is deg_hi else load_hi
            do = other[a] - other[b]
            blk[a], blk[b] = bmin, bmax
            slot[a], slot[b] = slot[b], slot[a]
            load[bmax] -= d
            load[bmin] += d
            oload[bmax] -= do
            oload[bmin] += do

    gslot = blk * 128 + slot  # node -> padded global row

    # --- group edges by (dest block, lo/hi section) ---
    be = blk[rows]
    hi = (cols >= LOHI).astype(np.int64)
    key = be * 2 + hi
    order_e = np.argsort(key, kind="stable")
    be_s = be[order_e]
    hi_s = hi[order_e]
    lloc_s = slot[rows[order_e]].astype(np.float32)
    qidx_s = (be_s % NB) * 128 + slot[rows[order_e]]
    cidx_s = cols[order_e]

    counts = np.bincount(key, minlength=NBLK * 2).reshape(NBLK, 2)
    tlo = max(1, int(np.ceil(counts[:, 0].max() / 128)))
    thi = int(np.ceil(counts[:, 1].max() / 128)) if N > LOHI else 0
    t_b = tlo + thi
    cap = t_b * 128

    # lane of each edge within its block: lo edges from 0, hi from tlo*128
    sec_start = np.array([0, tlo * 128], dtype=np.int64)
    starts = np.zeros(NBLK * 2, dtype=np.int64)
    np.cumsum(counts.reshape(-1)[:-1], out=starts[1:])
    pos_e = np.arange(E, dtype=np.int64) - starts[key[order_e]]
    lane = pos_e + sec_start[hi_s]
    slot_e = be_s * cap + lane

    nslots = NBLK * cap
    lloc_a = np.full(nslots, -1.0, dtype=np.float32)
    qidx_a = np.zeros(nslots, dtype=np.int64)
    cidx_a = np.zeros(nslots, dtype=np.int64)
    lloc_a[slot_e] = lloc_s
    qidx_a[slot_e] = qidx_s
    cidx_a[slot_e] = cidx_s

    lloc_a = lloc_a.reshape(NBLK, t_b, 128)
    qidx_a = qidx_a.reshape(NBLK, cap)
    cidx_a = cidx_a.reshape(NBLK, cap)

    ilo_w = _wrap16(cidx_a[:, : tlo * 128], NBLK, tlo)
    if thi > 0:
        chi = np.maximum(cidx_a[:, tlo * 128 :] - LOHI, 0)  # pads hold col 0
        ihi_w = _wrap16(chi, NBLK, thi)
    else:
        ihi_w = np.zeros((NBLK * 128, 0), np.int16)
    iq_w = _wrap16(qidx_a, NBLK, t_b)

    # lloc: [NBLK, t_b, 128] -> [NBLK*128, t_b] (partition p = lane), bf16 bits
    import ml_dtypes

    lloc_w = (
        np.ascontiguousarray(lloc_a.transpose(0, 2, 1))
        .reshape(NBLK * 128, t_b)
        .astype(ml_dtypes.bfloat16)
        .view(np.int16)
    )

    # merged per-block int16 row: [lloc | ilo | ihi | iq]
    wm = t_b + tlo * 8 + thi * 8 + t_b * 8
    idxs_w = np.concatenate([lloc_w, ilo_w, ihi_w, iq_w], axis=1).reshape(
        NCORES, NBP, wm
    )

    embeds = np.ascontiguousarray(np.asarray(embeds, dtype=np.float32))
    emb_pad = np.zeros((NCORES * NBP, D), dtype=np.float32)
    emb_pad[gslot] = embeds
    emb_pad = emb_pad.reshape(NCORES, NBP, D)

    qTrans = np.ascontiguousarray(np.asarray(qTrans, dtype=np.float32))
    kTrans = np.ascontiguousarray(np.asarray(kTrans, dtype=np.float32))
    vTrans = np.ascontiguousarray(np.asarray(vTrans, dtype=np.float32))
    ln_scale = np.ascontiguousarray(np.asarray(ln_scale, dtype=np.float32))
    ln_bias = np.ascontiguousarray(np.asarray(ln_bias, dtype=np.float32))

    in_maps = []
    for c in range(NCORES):
        in_maps.append(
            {
                "embeds": embeds,
                "emb_local": emb_pad[c],
                "qT": qTrans,
                "kT": kTrans,
                "vT": vTrans,
                "lnsc": ln_scale,
                "lnb": ln_bias,
                "idxs": np.ascontiguousarray(idxs_w[c]),
            }
        )
    return in_maps, (t_b, tlo), gslot


_PROGRAM_CACHE: dict[tuple, bass.Bass] = {}


def kernel(embeds, edge_index, qTrans, kTrans, vTrans, ln_scale, ln_bias, **_):
    in_maps, key, gslot = _prepare_core_inputs(
        embeds, edge_index, qTrans, kTrans, vTrans, ln_scale, ln_bias
    )
    nc = _PROGRAM_CACHE.get(key)
    if nc is None:
        nc = build_program(*key)
        _PROGRAM_CACHE[key] = nc

    res = run_bass_kernel_spmd(nc, in_maps, core_ids=list(range(NCORES)))
    all_out = np.concatenate([res.results[c]["out"] for c in range(NCORES)], axis=0)
    return np.ascontiguousarray(all_out[gslot])


if __name__ == "__main__":
    rng = np.random.default_rng(0)
    inputs = {
        "embeds": rng.standard_normal((N, D), dtype=np.float32),
        "edge_index": rng.integers(0, N, size=(2, E)).astype(np.int64),
        "qTrans": (rng.standard_normal((D, D), dtype=np.float32) / np.sqrt(D)).astype(
            np.float32
        ),
        "kTrans": (rng.standard_normal((D, D), dtype=np.float32) / np.sqrt(D)).astype(
            np.float32
        ),
        "vTrans": (rng.standard_normal((D, D), dtype=np.float32) / np.sqrt(D)).astype(
            np.float32
        ),
        "ln_scale": np.ones(D, dtype=np.float32),
        "ln_bias": np.zeros(D, dtype=np.float32),
    }
    out = kernel(**inputs)
    print("kernel output", out.shape, out.dtype, np.isfinite(out).all())

